# revision 42
# baseline (speedup 1.0000x reference)
"""Causal attention block (B=4, S=2048, D=1024, H=16) on 8 Trainium2 NeuronCores.

Sharding: core c = (batch b = c//2, head-group hg = c%2 of 8 heads).
Each core computes QKV projection for its batch restricted to its heads'
columns, causal flash-style attention for its 8 heads, and a partial output
projection (its heads' rows of W_proj). Host sums the two partial outputs
per batch pair and returns the full [4, 2048, 1024] result.

All matmul operands are bf16 (PSUM accumulation stays fp32): on TRN2 the PE
processes one moving row per cycle regardless of dtype, but bf16 halves DMA
bytes, lifts the fp32r moving<256 penalty, and doubles 2-byte DVE ops.

Engine balance: projections (QKV + output) are PE-bound; attention is
Activation-bound (the exp chain). Attention starts as soon as the first
head's q/k columns and the first four key blocks of v exist; every other
projection group lives in an ordered filler queue drained one group per
score-pair iteration (with forced draining to satisfy data dependencies), so
the PE chews projection work whenever the scalar engine is the attention
rate limiter and ideally never idles.

Layout choices (per core):
  - x arrives pre-transposed as xT [1024, 2048] so the embedding dim (the
    matmul contraction dim) is the SBUF partition dim.
  - q, k are produced transposed: qT/kT [512 cols, 2048 tokens] stored as
    [128, 4, 2048] tiles; head h lives in tile chunk h//2, partitions
    (h%2)*64..+64. 1/sqrt(hd) folded into W_q on the host.
  - v is produced in natural [token, col] orientation as [128, 16, 8, 65]
    (key-block, head, 64 v-cols + a ones column for softmax denominators).
  - scores are computed transposed, sT[k, q] = kT_block.T @ qT, into paired
    [128, 2, 512] PSUM tiles so one exp covers two full key blocks (halving
    the activation-engine per-instruction overhead), exp'd with no max
    subtraction (scores are ~N(0,1); fp32 exp cannot overflow), causal
    diagonal masked by a triangular multiply.
  - attention output accumulates in the efficient o[q, d] orientation
    (lhsT = es[k, q-subblock 128], rhs = v[k, 65]): stationary = 128 queries,
    moving = 65, i.e. half the PE rows of the oT[d, q] orientation. The ones
    column yields the denominator as o[:, 64]. PSUM start=True zeroes the
    whole 2KB bank, so only the first write into each bank sets it.
  - normalization is a per-partition tensor_scalar multiply by the
    reciprocal denominator (no partition broadcast needed), packing head
    pairs side by side; a PE transpose of [128, 128] blocks then restores the
    oT[c, q] layout the output projection needs as lhsT.
"""

from collections import deque

import numpy as np
import ml_dtypes

import concourse.bass as bass
import concourse.mybir as mybir
import concourse.tile as tile
from concourse import bacc
from concourse.bass_utils import run_bass_kernel_spmd
from concourse.masks import make_upper_triangular

F32 = mybir.dt.float32
BF16 = mybir.dt.bfloat16
EMB = 1024
HEADS = 16
HD = 64
B = 4
S = 2048
NCORES = 8
HPC = 8           # heads per core
CD = HPC * HD     # 512 cols per core for each of q/k/v
NKB = S // 128    # 16 key blocks
NQC = S // 512    # 4 query chunks

_EXP = mybir.ActivationFunctionType.Exp
_COPY = mybir.ActivationFunctionType.Copy


def _build_module():
    nc = bacc.Bacc("TRN2", target_bir_lowering=False, debug=False)
    xT = nc.declare_dram_parameter("xT", [EMB, S], BF16, isOutput=False)
    wq = nc.declare_dram_parameter("wq", [EMB, CD], BF16, isOutput=False)
    wk = nc.declare_dram_parameter("wk", [EMB, CD], BF16, isOutput=False)
    wv = nc.declare_dram_parameter("wv", [EMB, CD], BF16, isOutput=False)
    wp = nc.declare_dram_parameter("wp", [CD, EMB], BF16, isOutput=False)
    bias = nc.declare_dram_parameter("bias", [1, EMB], BF16, isOutput=False)
    ident = nc.declare_dram_parameter("ident", [128, 128], BF16, isOutput=False)
    y = nc.declare_dram_parameter("y", [S, EMB], BF16, isOutput=True)

    with tile.TileContext(nc) as tc:
        _body(tc, nc, xT, wq, wk, wv, wp, bias, ident, y)
    nc.compile()
    return nc


def _body(tc, nc, xT, wq, wk, wv, wp, bias, ident, y):
    from contextlib import ExitStack

    with ExitStack() as ctx:
        persist = ctx.enter_context(tc.tile_pool(name="persist", bufs=1))
        qt = persist.tile([128, 4, S], BF16, tag="qt")
        kt = persist.tile([128, 4, S], BF16, tag="kt")
        vx = persist.tile([128, NKB, HPC, HD + 1], BF16, tag="vx")
        tri = persist.tile([128, 128], BF16, tag="tri")
        ident_sb = persist.tile([128, 128], BF16, tag="ident")
        wp_sb = persist.tile([128, 4, EMB], BF16, tag="wp")
        bias_sb = persist.tile([128, 1, EMB], BF16, tag="bias")

        # ones column for denominators; causal tri[p, f] = 1.0 iff f >= p
        nc.gpsimd.memset(vx[:, :, :, HD : HD + 1], 1.0)
        make_upper_triangular(nc, tri[:], val=1.0, diag=True)

        xt_pool = ctx.enter_context(tc.tile_pool(name="xt", bufs=2))
        w_pool = ctx.enter_context(tc.tile_pool(name="w", bufs=8))
        wv_pool = ctx.enter_context(tc.tile_pool(name="wvp", bufs=1))
        mm_ps = ctx.enter_context(tc.tile_pool(name="mmps", bufs=2, space="PSUM"))
        s_pool = ctx.enter_context(tc.tile_pool(name="sps", bufs=2, space="PSUM"))
        o_pool = ctx.enter_context(tc.tile_pool(name="ops", bufs=1, space="PSUM"))
        oT_ps_pool = ctx.enter_context(
            tc.tile_pool(name="oTps", bufs=1, space="PSUM")
        )
        e_pool = ctx.enter_context(tc.tile_pool(name="es", bufs=4))
        r_pool = ctx.enter_context(tc.tile_pool(name="recip", bufs=2))
        pair_pool = ctx.enter_context(tc.tile_pool(name="pair", bufs=2))
        oT_pool = ctx.enter_context(tc.tile_pool(name="oT", bufs=2))
        ysb_pool = ctx.enter_context(tc.tile_pool(name="ysb", bufs=4))

        # ---- input loads, spread across the four DMA-issuing engines ----
        # SP: all xT tiles (first-needed first). Act: wq tiles. DVE: wk tiles.
        # Pool: wv, then constants needed only later (ident/wp/bias).
        xt_tiles = {
            0: xt_pool.tile([128, 8, 1024], BF16, tag="xt", name="xt0"),
            1: xt_pool.tile([128, 8, 1024], BF16, tag="xt", name="xt1"),
        }

        def load_xt(half, n2, engine):
            xt_sb = xt_tiles[half]
            for kc in range(8):
                c0 = half * 1024 + n2 * 512
                engine.dma_start(
                    out=xt_sb[:, kc, n2 * 512 : (n2 + 1) * 512],
                    in_=xT[kc * 128 : (kc + 1) * 128, c0 : c0 + 512],
                )

        w_tiles = {}

        def load_w(qk, m, eng, split=False):
            wdram = wq if qk == 0 else wk
            wt = w_pool.tile([128, 8, 128], BF16, tag="w", name=f"w{qk}{m}")
            halves = ((0, 4), (4, 8)) if split else ((0, 8),)
            for c0, c1 in halves:
                eng.dma_start(
                    out=wt[:, c0:c1, :],
                    in_=wdram[
                        c0 * 128 : c1 * 128, m * 128 : (m + 1) * 128
                    ].rearrange("(c p) m -> p c m", p=128),
                )
            w_tiles[(qk, m)] = wt

        # startup: first x quarter split across SP and Act queues, the Act
        # half interleaved with the first head-pairs' projection weights
        xt_sb0 = xt_tiles[0]
        for kc in range(4):
            nc.sync.dma_start(
                out=xt_sb0[:, kc, 0:512], in_=xT[kc * 128 : (kc + 1) * 128, 0:512]
            )
        load_w(0, 0, nc.scalar, split=True)
        for kc in range(4, 6):
            nc.scalar.dma_start(
                out=xt_sb0[:, kc, 0:512], in_=xT[kc * 128 : (kc + 1) * 128, 0:512]
            )
        load_w(1, 0, nc.scalar, split=True)
        for kc in range(6, 8):
            nc.scalar.dma_start(
                out=xt_sb0[:, kc, 0:512], in_=xT[kc * 128 : (kc + 1) * 128, 0:512]
            )
        load_w(0, 1, nc.scalar)
        load_w(1, 1, nc.scalar)
        wv_sb = wv_pool.tile([128, 8, CD], BF16, tag="wv")
        for kc in range(8):
            nc.gpsimd.dma_start(
                out=wv_sb[:, kc, :], in_=wv[kc * 128 : (kc + 1) * 128, :]
            )
        for m in range(2, 4):
            load_w(0, m, nc.scalar)
            load_w(1, m, nc.scalar)
        load_xt(0, 1, nc.gpsimd)
        load_xt(1, 0, nc.sync)
        load_xt(1, 1, nc.sync)
        nc.gpsimd.dma_start(out=ident_sb[:], in_=ident[:])
        nc.gpsimd.dma_start(
            out=wp_sb[:], in_=wp[:].rearrange("(c p) e -> p c e", p=128)
        )
        nc.gpsimd.dma_start(out=bias_sb[:], in_=bias[:].partition_broadcast(128))

        # ---- projection group emitters ----
        def qk_group(half, qk, m, n):
            xt_sb = xt_tiles[half]
            wt = w_tiles[(qk, m)]
            dst = qt if qk == 0 else kt
            ps = mm_ps.tile([128, 512], F32, tag="mmps", name="qkps")
            for kc in range(8):
                nc.tensor.matmul(
                    ps[:],
                    lhsT=(wt[:, kc, :]),
                    rhs=(xt_sb[:, kc, n * 512 : (n + 1) * 512]),
                    start=(kc == 0),
                    stop=(kc == 7),
                )
            col = half * 1024 + n * 512
            nc.vector.tensor_copy(out=dst[:, m, col : col + 512], in_=ps[:])

        def v_group(half, tc8):
            xt_sb = xt_tiles[half]
            tg = half * 8 + tc8
            ps = mm_ps.tile([128, 512], F32, tag="mmps", name="vps")
            for kc in range(8):
                nc.tensor.matmul(
                    ps[:],
                    lhsT=(xt_sb[:, kc, tc8 * 128 : (tc8 + 1) * 128]),
                    rhs=(wv_sb[:, kc, :]),
                    start=(kc == 0),
                    stop=(kc == 7),
                )
            nc.vector.tensor_copy(
                out=vx[:, tg, :, 0:HD],
                in_=ps[:].rearrange("p (h d) -> p h d", h=HPC),
            )

        def make_y_group(oT_prev, qc_prev, tc4, ncol):
            def emit():
                row = qc_prev * 512 + tc4 * 128
                y_ps = mm_ps.tile([128, 512], F32, tag="mmps", name="yps")
                tail = qc_prev == NQC - 1
                for kc in range(4):
                    nc.tensor.matmul(
                        y_ps[:],
                        lhsT=(oT_prev[:, kc, tc4 * 128 : (tc4 + 1) * 128]),
                        rhs=(wp_sb[:, kc, ncol * 512 : (ncol + 1) * 512]),
                        start=(kc == 0),
                        stop=(kc == 3 and not tail),
                        skip_group_check=tail,
                    )
                y_sb = ysb_pool.tile([128, 512], BF16, tag="ysb", name="ysb")
                if qc_prev == NQC - 1:
                    # tail: bias via a 1-partition PE matmul (tri row 0 is
                    # all ones) and copy on the idle Act engine; the DVE
                    # would otherwise serialize the kernel tail
                    nc.tensor.matmul(
                        y_ps[:],
                        lhsT=tri[0:1, :],
                        rhs=bias_sb[0:1, 0, ncol * 512 : (ncol + 1) * 512],
                        start=False,
                        stop=True,
                        skip_group_check=True,
                    )
                    nc.scalar.activation(
                        out=y_sb[:], in_=y_ps[:], func=_COPY
                    )
                else:
                    nc.vector.tensor_add(
                        y_sb[:],
                        y_ps[:],
                        bias_sb[:, 0, ncol * 512 : (ncol + 1) * 512],
                    )
                # last chunk's stores drain at the kernel tail: alternate
                # queues so the final DMAs overlap instead of serializing
                eng = (
                    nc.scalar
                    if qc_prev == NQC - 1 and (2 * tc4 + ncol) % 2
                    else nc.sync
                )
                eng.dma_start(
                    out=y[row : row + 128, ncol * 512 : (ncol + 1) * 512],
                    in_=y_sb[:],
                )

            return emit

        # Ordered projection-group queue: q/k token-slice groups (m-ascending
        # per chunk so heads unblock progressively). gate[(qc, m)] = count
        # that must be emitted before attention chunk qc head-pair m may run.
        # Paced (voluntary) pops are capped below the last chunk's section so
        # that work remains to fill the PE during the Act-bound last chunk.
        # Deferred output projections go to a second queue popped on the
        # pacing slots. v groups are drained at AV-emission granularity.
        G = []
        gate = {}
        for qc in range(NQC):
            half, n = qc // 2, qc % 2
            for m in range(4):
                G.append(("qk", half, 0, m, n))
                G.append(("qk", half, 1, m, n))
                gate[(qc, m)] = len(G)
        gq = deque(G)
        yq = deque()
        drained = [0]
        cur_qc = [0]
        vq = deque((kb // 8, kb % 8) for kb in range(NKB))
        v_drained = [0]

        def pop_gq():
            item = gq.popleft()
            qk_group(item[1], item[2], item[3], item[4])
            drained[0] += 1

        debt = [0.0]  # emitted Act-ns minus emitted PE-ns (cost model est.)

        def pop_filler(reserve=0, uncap=False):
            # per-chunk filler balance: chunk qc's attention consumes the
            # NEXT chunk's q/k groups as filler (the last chunk's are held
            # for its own Act-heavy stretch), and output projections are
            # held until two chunks after they were produced
            qc = cur_qc[0]
            if qc >= 2 and len(yq) > reserve:
                yq.popleft()()
                debt[0] -= 853.0
                return True
            if gq and (uncap or drained[0] < gate[(min(qc + 1, 2), 3)]):
                pop_gq()
                debt[0] -= 1707.0
                return True
            return False

        def pop_while_indebted():
            # keep the PE's emitted work level with the Act engine's: pop
            # filler until the modeled activation debt is covered
            while debt[0] > -1500 and pop_filler(reserve=0, uncap=(cur_qc[0] == NQC - 1)):
                pass

        def drain_to(idx):
            while drained[0] < idx:
                pop_gq()

        def drain_v_to(kb_hi):
            while v_drained[0] <= kb_hi and vq:
                half, tc8 = vq.popleft()
                v_group(half, tc8)
                v_drained[0] += 1
                debt[0] -= 1707.0

        # ---------------- attention (Act-bound) + filler drain ----------------
        it = [0]
        for qc in range(NQC):
            cur_qc[0] = qc
            oT = oT_pool.tile([128, 4, 512], BF16, tag="oT")
            kb_max = 4 * qc + 4
            o_sb = None
            for h in range(HPC):
                m, hh = h // 2, h % 2
                drain_to(gate[(qc, m)])
                o_ps = o_pool.tile([128, 4, 128], F32, tag="ops")

                def emit_av(kb, q0, es, jj):
                    sb0 = q0 // 128
                    for sb in range(sb0, 4):
                        nc.tensor.matmul(
                            out=o_ps[:, sb, 0 : HD + 1],
                            lhsT=(
                                es[:, jj, (sb - sb0) * 128 : (sb - sb0 + 1) * 128]
                            ),
                            rhs=(vx[:, kb, h, :]),
                            start=(kb == 0 and sb == 0),
                            stop=(kb == 4 * qc + sb),
                            skip_group_check=True,
                        )

                pending = []
                for pj in range(kb_max // 2):
                    it[0] += 1
                    s_ps = s_pool.tile([128, 2, 512], F32, tag="s")
                    es = e_pool.tile([128, 2, 512], BF16, tag="es")
                    nqs = []
                    for jj in range(2):
                        kb = 2 * pj + jj
                        r = kb * 128 - qc * 512
                        q0 = max(r, 0)
                        nq = 512 - q0
                        nqs.append((kb, q0, nq))
                        nc.tensor.matmul(
                            out=s_ps[:, jj, 0:nq],
                            lhsT=(
                                kt[hh * 64 : hh * 64 + 64, m, kb * 128 : (kb + 1) * 128]
                            ),
                            rhs=(
                                qt[
                                    hh * 64 : hh * 64 + 64,
                                    m,
                                    qc * 512 + q0 : (qc + 1) * 512,
                                ]
                            ),
                            start=True,
                            stop=True,
                        )
                    if nqs[0][2] == 512 and nqs[1][2] == 512:
                        # full pair: one exp over both banks
                        nc.scalar.activation(out=es[:], in_=s_ps[:], func=_EXP)
                        debt[0] += 1024 * 0.833 + 185
                    else:
                        for jj, (kb, q0, nq) in enumerate(nqs):
                            nc.scalar.activation(
                                out=es[:, jj, 0:nq], in_=s_ps[:, jj, 0:nq], func=_EXP
                            )
                            debt[0] += nq * 0.833 + 185
                    debt[0] -= (nqs[0][2] + nqs[1][2]) * 0.4167  # scores
                    for jj, (kb, q0, nq) in enumerate(nqs):
                        if kb * 128 - qc * 512 >= 0:
                            # diagonal block: mask the first 128 query columns
                            nc.vector.tensor_mul(
                                es[:, jj, 0:128], es[:, jj, 0:128], tri[:]
                            )
                    # av matmuls run one pair behind so the PE never waits on
                    # the exp of the pair it just produced
                    if pending:
                        drain_v_to(pending[-1][0])
                    for kb, q0, nq in pending:
                        emit_av(kb, q0, es_prev, kb & 1)
                        debt[0] -= (4 - q0 // 128) * 65 * 0.4167
                    pending, es_prev = nqs, es
                    pop_while_indebted()
                if pending:
                    drain_v_to(pending[-1][0])
                for kb, q0, nq in pending:
                    emit_av(kb, q0, es_prev, kb & 1)
                    debt[0] -= (4 - q0 // 128) * 65 * 0.4167
                pop_while_indebted()
                # normalize: per-partition multiply by 1/denominator
                recip = r_pool.tile([128, 4], F32, tag="recip")
                nc.vector.reciprocal(recip[:], o_ps[:, :, HD])
                if hh == 0:
                    o_sb = pair_pool.tile([128, 4, 128], BF16, tag="pair")
                for sb in range(4):
                    nc.vector.tensor_scalar_mul(
                        o_sb[:, sb, hh * 64 : hh * 64 + 64],
                        o_ps[:, sb, 0:HD],
                        recip[:, sb : sb + 1],
                    )
                if hh == 1:
                    # fill the PE while the normalize chain runs on the DVE;
                    # in the last chunk this may pull gated groups early
                    pop_filler(uncap=(qc == NQC - 1))
                    if m < 3:
                        # async XBAR DMA transposes the [128 q, 128 c] blocks
                        # into the oT[c, q] layout the projection needs --
                        # zero PE/DVE cost; the latency hides behind the rest
                        # of the chunk's attention
                        for sb in range(4):
                            nc.sync.dma_start_transpose(
                                out=oT[:, m, sb * 128 : (sb + 1) * 128],
                                in_=o_sb[:, sb, :],
                            )
                    else:
                        # last pair: its oT gates the next chunk's deferred
                        # output projections, so take the low-latency PE path
                        oT_ps = oT_ps_pool.tile([128, 4, 256], BF16, tag="oTps")
                        for sb in range(4):
                            nc.tensor.matmul(
                                oT_ps[:, sb, 0:128],
                                lhsT=o_sb[:, sb, :],
                                rhs=ident_sb[:],
                                is_transpose=True,
                                start=(sb == 0),
                                stop=(sb == 3),
                                skip_group_check=True,
                            )
                        nc.vector.tensor_copy(
                            out=oT[:, m, :].rearrange("p (s q) -> p s q", s=4),
                            in_=oT_ps[:, :, 0:128],
                        )

            # defer this chunk's output projection into the filler queue
            for tc4 in range(4):
                for ncol in range(2):
                    yq.append(make_y_group(oT, qc, tc4, ncol))
        while yq or gq:
            if yq:
                yq.popleft()()
            else:
                pop_gq()


_MODULE = None


def _get_module():
    global _MODULE
    if _MODULE is None:
        _MODULE = _build_module()
    return _MODULE


def _bf16(a):
    return np.ascontiguousarray(np.asarray(a, dtype=np.float32)).astype(
        ml_dtypes.bfloat16
    )


def _make_in_maps(x, W_qkv, W_proj, b_proj):
    scale = np.float32(1.0 / np.sqrt(HD))
    bias_half = (np.asarray(b_proj, dtype=np.float32) * 0.5).reshape(1, EMB)
    in_maps = []
    for c in range(NCORES):
        b, hg = c // 2, c % 2
        cols = slice(hg * CD, (hg + 1) * CD)
        in_maps.append(
            {
                "xT": _bf16(np.asarray(x[b], dtype=np.float32).T),
                "wq": _bf16(W_qkv[:, 0:EMB][:, cols] * scale),
                "wk": _bf16(W_qkv[:, EMB : 2 * EMB][:, cols]),
                "wv": _bf16(W_qkv[:, 2 * EMB : 3 * EMB][:, cols]),
                "wp": _bf16(W_proj[cols, :]),
                "bias": _bf16(bias_half),
                "ident": np.eye(128, dtype=ml_dtypes.bfloat16),
            }
        )
    return in_maps


def kernel(x, W_qkv, W_proj, b_proj, _trace=False, _trace_kwargs=None):
    x = np.asarray(x, dtype=np.float32)
    W_qkv = np.asarray(W_qkv, dtype=np.float32)
    W_proj = np.asarray(W_proj, dtype=np.float32)
    b_proj = np.asarray(b_proj, dtype=np.float32)

    nc = _get_module()
    in_maps = _make_in_maps(x, W_qkv, W_proj, b_proj)
    res = run_bass_kernel_spmd(
        nc, in_maps, list(range(NCORES)), trace=_trace, **(_trace_kwargs or {})
    )
    out = np.empty((B, S, EMB), dtype=np.float32)
    for b in range(B):
        out[b] = np.asarray(res.results[2 * b]["y"], dtype=np.float32) + np.asarray(
            res.results[2 * b + 1]["y"], dtype=np.float32
        )
    if _trace:
        return out, res
    return out


# revision 44
# speedup vs baseline: 1.0007x; 1.0007x over previous
"""Causal attention block (B=4, S=2048, D=1024, H=16) on 8 Trainium2 NeuronCores.

Sharding: core c = (batch b = c//2, head-group hg = c%2 of 8 heads).
Each core computes QKV projection for its batch restricted to its heads'
columns, causal flash-style attention for its 8 heads, and a partial output
projection (its heads' rows of W_proj). Host sums the two partial outputs
per batch pair and returns the full [4, 2048, 1024] result.

All matmul operands are bf16 (PSUM accumulation stays fp32): on TRN2 the PE
processes one moving row per cycle regardless of dtype, but bf16 halves DMA
bytes, lifts the fp32r moving<256 penalty, and doubles 2-byte DVE ops.

Engine balance: projections (QKV + output) are PE-bound; attention is
Activation-bound (the exp chain). Attention starts as soon as the first
head's q/k columns and the first four key blocks of v exist; every other
projection group lives in an ordered filler queue drained one group per
score-pair iteration (with forced draining to satisfy data dependencies), so
the PE chews projection work whenever the scalar engine is the attention
rate limiter and ideally never idles.

Layout choices (per core):
  - x arrives pre-transposed as xT [1024, 2048] so the embedding dim (the
    matmul contraction dim) is the SBUF partition dim.
  - q, k are produced transposed: qT/kT [512 cols, 2048 tokens] stored as
    [128, 4, 2048] tiles; head h lives in tile chunk h//2, partitions
    (h%2)*64..+64. 1/sqrt(hd) folded into W_q on the host.
  - v is produced in natural [token, col] orientation as [128, 16, 8, 65]
    (key-block, head, 64 v-cols + a ones column for softmax denominators).
  - scores are computed transposed, sT[k, q] = kT_block.T @ qT, into paired
    [128, 2, 512] PSUM tiles so one exp covers two full key blocks (halving
    the activation-engine per-instruction overhead), exp'd with no max
    subtraction (scores are ~N(0,1); fp32 exp cannot overflow), causal
    diagonal masked by a triangular multiply.
  - attention output accumulates in the efficient o[q, d] orientation
    (lhsT = es[k, q-subblock 128], rhs = v[k, 65]): stationary = 128 queries,
    moving = 65, i.e. half the PE rows of the oT[d, q] orientation. The ones
    column yields the denominator as o[:, 64]. PSUM start=True zeroes the
    whole 2KB bank, so only the first write into each bank sets it.
  - normalization is a per-partition tensor_scalar multiply by the
    reciprocal denominator (no partition broadcast needed), packing head
    pairs side by side; a PE transpose of [128, 128] blocks then restores the
    oT[c, q] layout the output projection needs as lhsT.
"""

from collections import deque

import numpy as np
import ml_dtypes

import concourse.bass as bass
import concourse.mybir as mybir
import concourse.tile as tile
from concourse import bacc
from concourse.bass_utils import run_bass_kernel_spmd
from concourse.masks import make_upper_triangular

F32 = mybir.dt.float32
BF16 = mybir.dt.bfloat16
EMB = 1024
HEADS = 16
HD = 64
B = 4
S = 2048
NCORES = 8
HPC = 8           # heads per core
CD = HPC * HD     # 512 cols per core for each of q/k/v
NKB = S // 128    # 16 key blocks
NQC = S // 512    # 4 query chunks

_EXP = mybir.ActivationFunctionType.Exp
_COPY = mybir.ActivationFunctionType.Copy


def _build_module():
    nc = bacc.Bacc("TRN2", target_bir_lowering=False, debug=False)
    xT = nc.declare_dram_parameter("xT", [EMB, S], BF16, isOutput=False)
    wq = nc.declare_dram_parameter("wq", [EMB, CD], BF16, isOutput=False)
    wk = nc.declare_dram_parameter("wk", [EMB, CD], BF16, isOutput=False)
    wv = nc.declare_dram_parameter("wv", [EMB, CD], BF16, isOutput=False)
    wp = nc.declare_dram_parameter("wp", [CD, EMB], BF16, isOutput=False)
    bias = nc.declare_dram_parameter("bias", [1, EMB], BF16, isOutput=False)
    ident = nc.declare_dram_parameter("ident", [128, 128], BF16, isOutput=False)
    y = nc.declare_dram_parameter("y", [S, EMB], BF16, isOutput=True)

    with tile.TileContext(nc) as tc:
        _body(tc, nc, xT, wq, wk, wv, wp, bias, ident, y)
    nc.compile()
    return nc


def _body(tc, nc, xT, wq, wk, wv, wp, bias, ident, y):
    from contextlib import ExitStack

    with ExitStack() as ctx:
        persist = ctx.enter_context(tc.tile_pool(name="persist", bufs=1))
        qt = persist.tile([128, 4, S], BF16, tag="qt")
        kt = persist.tile([128, 4, S], BF16, tag="kt")
        vx = persist.tile([128, NKB, HPC, HD + 1], BF16, tag="vx")
        tri = persist.tile([128, 128], BF16, tag="tri")
        ident_sb = persist.tile([128, 128], BF16, tag="ident")
        wp_sb = persist.tile([128, 4, EMB], BF16, tag="wp")
        bias_sb = persist.tile([128, 1, EMB], BF16, tag="bias")

        # ones column for denominators; causal tri[p, f] = 1.0 iff f >= p
        nc.gpsimd.memset(vx[:, :, :, HD : HD + 1], 1.0)
        make_upper_triangular(nc, tri[:], val=1.0, diag=True)

        xt_pool = ctx.enter_context(tc.tile_pool(name="xt", bufs=2))
        w_pool = ctx.enter_context(tc.tile_pool(name="w", bufs=8))
        wv_pool = ctx.enter_context(tc.tile_pool(name="wvp", bufs=1))
        mm_ps = ctx.enter_context(tc.tile_pool(name="mmps", bufs=2, space="PSUM"))
        s_pool = ctx.enter_context(tc.tile_pool(name="sps", bufs=2, space="PSUM"))
        o_pool = ctx.enter_context(tc.tile_pool(name="ops", bufs=1, space="PSUM"))
        oT_ps_pool = ctx.enter_context(
            tc.tile_pool(name="oTps", bufs=1, space="PSUM")
        )
        e_pool = ctx.enter_context(tc.tile_pool(name="es", bufs=4))
        r_pool = ctx.enter_context(tc.tile_pool(name="recip", bufs=2))
        pair_pool = ctx.enter_context(tc.tile_pool(name="pair", bufs=2))
        oT_pool = ctx.enter_context(tc.tile_pool(name="oT", bufs=2))
        ysb_pool = ctx.enter_context(tc.tile_pool(name="ysb", bufs=4))

        # ---- input loads, spread across the four DMA-issuing engines ----
        # SP: all xT tiles (first-needed first). Act: wq tiles. DVE: wk tiles.
        # Pool: wv, then constants needed only later (ident/wp/bias).
        xt_tiles = {
            0: xt_pool.tile([128, 8, 1024], BF16, tag="xt", name="xt0"),
            1: xt_pool.tile([128, 8, 1024], BF16, tag="xt", name="xt1"),
        }

        def load_xt(half, n2, engine):
            xt_sb = xt_tiles[half]
            for kc in range(8):
                c0 = half * 1024 + n2 * 512
                engine.dma_start(
                    out=xt_sb[:, kc, n2 * 512 : (n2 + 1) * 512],
                    in_=xT[kc * 128 : (kc + 1) * 128, c0 : c0 + 512],
                )

        w_tiles = {}

        def load_w(qk, m, eng, split=False):
            wdram = wq if qk == 0 else wk
            wt = w_pool.tile([128, 8, 128], BF16, tag="w", name=f"w{qk}{m}")
            halves = ((0, 4), (4, 8)) if split else ((0, 8),)
            for c0, c1 in halves:
                eng.dma_start(
                    out=wt[:, c0:c1, :],
                    in_=wdram[
                        c0 * 128 : c1 * 128, m * 128 : (m + 1) * 128
                    ].rearrange("(c p) m -> p c m", p=128),
                )
            w_tiles[(qk, m)] = wt

        # startup: first x quarter split across SP and Act queues, the Act
        # half interleaved with the first head-pairs' projection weights
        xt_sb0 = xt_tiles[0]
        for kc in range(4):
            nc.sync.dma_start(
                out=xt_sb0[:, kc, 0:512], in_=xT[kc * 128 : (kc + 1) * 128, 0:512]
            )
        load_w(0, 0, nc.scalar, split=True)
        for kc in range(4, 6):
            nc.scalar.dma_start(
                out=xt_sb0[:, kc, 0:512], in_=xT[kc * 128 : (kc + 1) * 128, 0:512]
            )
        load_w(1, 0, nc.scalar, split=True)
        for kc in range(6, 8):
            nc.scalar.dma_start(
                out=xt_sb0[:, kc, 0:512], in_=xT[kc * 128 : (kc + 1) * 128, 0:512]
            )
        load_w(0, 1, nc.scalar)
        load_w(1, 1, nc.scalar)
        wv_sb = wv_pool.tile([128, 8, CD], BF16, tag="wv")
        for kc in range(8):
            nc.gpsimd.dma_start(
                out=wv_sb[:, kc, :], in_=wv[kc * 128 : (kc + 1) * 128, :]
            )
        for m in range(2, 4):
            load_w(0, m, nc.scalar)
            load_w(1, m, nc.scalar)
        load_xt(0, 1, nc.gpsimd)
        load_xt(1, 0, nc.sync)
        load_xt(1, 1, nc.sync)
        nc.gpsimd.dma_start(out=ident_sb[:], in_=ident[:])
        nc.gpsimd.dma_start(
            out=wp_sb[:], in_=wp[:].rearrange("(c p) e -> p c e", p=128)
        )
        nc.gpsimd.dma_start(out=bias_sb[:], in_=bias[:].partition_broadcast(128))

        # ---- projection group emitters ----
        def qk_group(half, qk, m, n):
            xt_sb = xt_tiles[half]
            wt = w_tiles[(qk, m)]
            dst = qt if qk == 0 else kt
            ps = mm_ps.tile([128, 512], F32, tag="mmps", name="qkps")
            for kc in range(8):
                nc.tensor.matmul(
                    ps[:],
                    lhsT=(wt[:, kc, :]),
                    rhs=(xt_sb[:, kc, n * 512 : (n + 1) * 512]),
                    start=(kc == 0),
                    stop=(kc == 7),
                )
            col = half * 1024 + n * 512
            nc.vector.tensor_copy(out=dst[:, m, col : col + 512], in_=ps[:])

        def v_group(half, tc8):
            xt_sb = xt_tiles[half]
            tg = half * 8 + tc8
            ps = mm_ps.tile([128, 512], F32, tag="mmps", name="vps")
            for kc in range(8):
                nc.tensor.matmul(
                    ps[:],
                    lhsT=(xt_sb[:, kc, tc8 * 128 : (tc8 + 1) * 128]),
                    rhs=(wv_sb[:, kc, :]),
                    start=(kc == 0),
                    stop=(kc == 7),
                )
            nc.vector.tensor_copy(
                out=vx[:, tg, :, 0:HD],
                in_=ps[:].rearrange("p (h d) -> p h d", h=HPC),
            )

        def make_y_group(oT_prev, qc_prev, tc4, ncol):
            def emit():
                row = qc_prev * 512 + tc4 * 128
                y_ps = mm_ps.tile([128, 512], F32, tag="mmps", name="yps")
                tail = qc_prev == NQC - 1
                for kc in range(4):
                    nc.tensor.matmul(
                        y_ps[:],
                        lhsT=(oT_prev[:, kc, tc4 * 128 : (tc4 + 1) * 128]),
                        rhs=(wp_sb[:, kc, ncol * 512 : (ncol + 1) * 512]),
                        start=(kc == 0),
                        stop=(kc == 3 and not tail),
                        skip_group_check=tail,
                    )
                y_sb = ysb_pool.tile([128, 512], BF16, tag="ysb", name="ysb")
                if qc_prev == NQC - 1:
                    # tail: bias via a 1-partition PE matmul (tri row 0 is
                    # all ones) and copy on the idle Act engine; the DVE
                    # would otherwise serialize the kernel tail
                    nc.tensor.matmul(
                        y_ps[:],
                        lhsT=tri[0:1, :],
                        rhs=bias_sb[0:1, 0, ncol * 512 : (ncol + 1) * 512],
                        start=False,
                        stop=True,
                        skip_group_check=True,
                    )
                    nc.scalar.activation(
                        out=y_sb[:], in_=y_ps[:], func=_COPY
                    )
                else:
                    nc.vector.tensor_add(
                        y_sb[:],
                        y_ps[:],
                        bias_sb[:, 0, ncol * 512 : (ncol + 1) * 512],
                    )
                # last chunk's stores drain at the kernel tail: alternate
                # queues so the final DMAs overlap instead of serializing
                eng = (
                    nc.scalar
                    if qc_prev == NQC - 1 and (2 * tc4 + ncol) % 2
                    else nc.sync
                )
                eng.dma_start(
                    out=y[row : row + 128, ncol * 512 : (ncol + 1) * 512],
                    in_=y_sb[:],
                )

            return emit

        # Ordered projection-group queue: q/k token-slice groups (m-ascending
        # per chunk so heads unblock progressively). gate[(qc, m)] = count
        # that must be emitted before attention chunk qc head-pair m may run.
        # Paced (voluntary) pops are capped below the last chunk's section so
        # that work remains to fill the PE during the Act-bound last chunk.
        # Deferred output projections go to a second queue popped on the
        # pacing slots. v groups are drained at AV-emission granularity.
        G = []
        gate = {}
        for qc in range(NQC):
            half, n = qc // 2, qc % 2
            for m in range(4):
                G.append(("qk", half, 0, m, n))
                G.append(("qk", half, 1, m, n))
                gate[(qc, m)] = len(G)
        gq = deque(G)
        yq = deque()
        drained = [0]
        cur_qc = [0]
        vq = deque((kb // 8, kb % 8) for kb in range(NKB))
        v_drained = [0]

        def pop_gq():
            item = gq.popleft()
            qk_group(item[1], item[2], item[3], item[4])
            drained[0] += 1

        debt = [0.0]  # emitted Act-ns minus emitted PE-ns (cost model est.)

        def pop_filler(reserve=0, uncap=False):
            # per-chunk filler balance: chunk qc's attention consumes the
            # NEXT chunk's q/k groups as filler (the last chunk's are held
            # for its own Act-heavy stretch), and output projections are
            # held until two chunks after they were produced
            qc = cur_qc[0]
            if qc >= 2 and len(yq) > reserve:
                yq.popleft()()
                debt[0] -= 853.0
                return True
            if gq and (uncap or drained[0] < gate[(min(qc + 1, 2), 3)]):
                pop_gq()
                debt[0] -= 1707.0
                return True
            return False

        def pop_while_indebted():
            # keep the PE's emitted work level with the Act engine's: pop
            # filler until the modeled activation debt is covered
            while debt[0] > 0 and pop_filler(reserve=0, uncap=(cur_qc[0] == NQC - 1)):
                pass

        def drain_to(idx):
            while drained[0] < idx:
                pop_gq()

        def drain_v_to(kb_hi):
            while v_drained[0] <= kb_hi and vq:
                half, tc8 = vq.popleft()
                v_group(half, tc8)
                v_drained[0] += 1
                debt[0] -= 1707.0

        # ---------------- attention (Act-bound) + filler drain ----------------
        it = [0]
        for qc in range(NQC):
            cur_qc[0] = qc
            oT = oT_pool.tile([128, 4, 512], BF16, tag="oT")
            kb_max = 4 * qc + 4
            o_sb = None
            for h in range(HPC):
                m, hh = h // 2, h % 2
                drain_to(gate[(qc, m)])
                o_ps = o_pool.tile([128, 4, 128], F32, tag="ops")

                def emit_av(kb, q0, es, jj):
                    sb0 = q0 // 128
                    for sb in range(sb0, 4):
                        nc.tensor.matmul(
                            out=o_ps[:, sb, 0 : HD + 1],
                            lhsT=(
                                es[:, jj, (sb - sb0) * 128 : (sb - sb0 + 1) * 128]
                            ),
                            rhs=(vx[:, kb, h, :]),
                            start=(kb == 0 and sb == 0),
                            stop=(kb == 4 * qc + sb),
                            skip_group_check=True,
                        )

                pending = []
                for pj in range(kb_max // 2):
                    it[0] += 1
                    s_ps = s_pool.tile([128, 2, 512], F32, tag="s")
                    es = e_pool.tile([128, 2, 512], BF16, tag="es")
                    nqs = []
                    for jj in range(2):
                        kb = 2 * pj + jj
                        r = kb * 128 - qc * 512
                        q0 = max(r, 0)
                        nq = 512 - q0
                        nqs.append((kb, q0, nq))
                        nc.tensor.matmul(
                            out=s_ps[:, jj, 0:nq],
                            lhsT=(
                                kt[hh * 64 : hh * 64 + 64, m, kb * 128 : (kb + 1) * 128]
                            ),
                            rhs=(
                                qt[
                                    hh * 64 : hh * 64 + 64,
                                    m,
                                    qc * 512 + q0 : (qc + 1) * 512,
                                ]
                            ),
                            start=True,
                            stop=True,
                        )
                    if nqs[0][2] == 512 and nqs[1][2] == 512:
                        # full pair: one exp over both banks
                        nc.scalar.activation(out=es[:], in_=s_ps[:], func=_EXP)
                        debt[0] += 1024 * 0.833 + 185
                    else:
                        for jj, (kb, q0, nq) in enumerate(nqs):
                            nc.scalar.activation(
                                out=es[:, jj, 0:nq], in_=s_ps[:, jj, 0:nq], func=_EXP
                            )
                            debt[0] += nq * 0.833 + 185
                    debt[0] -= (nqs[0][2] + nqs[1][2]) * 0.4167  # scores
                    for jj, (kb, q0, nq) in enumerate(nqs):
                        if kb * 128 - qc * 512 >= 0:
                            # diagonal block: mask the first 128 query columns
                            nc.vector.tensor_mul(
                                es[:, jj, 0:128], es[:, jj, 0:128], tri[:]
                            )
                    # av matmuls run one pair behind so the PE never waits on
                    # the exp of the pair it just produced
                    if pending:
                        drain_v_to(pending[-1][0])
                    for kb, q0, nq in pending:
                        emit_av(kb, q0, es_prev, kb & 1)
                        debt[0] -= (4 - q0 // 128) * 65 * 0.4167
                    pending, es_prev = nqs, es
                    pop_while_indebted()
                if pending:
                    drain_v_to(pending[-1][0])
                for kb, q0, nq in pending:
                    emit_av(kb, q0, es_prev, kb & 1)
                    debt[0] -= (4 - q0 // 128) * 65 * 0.4167
                pop_while_indebted()
                # normalize: per-partition multiply by 1/denominator
                recip = r_pool.tile([128, 4], F32, tag="recip")
                nc.vector.reciprocal(recip[:], o_ps[:, :, HD])
                if hh == 0:
                    o_sb = pair_pool.tile([128, 4, 128], BF16, tag="pair")
                for sb in range(4):
                    nc.vector.tensor_scalar_mul(
                        o_sb[:, sb, hh * 64 : hh * 64 + 64],
                        o_ps[:, sb, 0:HD],
                        recip[:, sb : sb + 1],
                    )
                if hh == 1:
                    # fill the PE while the normalize chain runs on the DVE;
                    # in the last chunk this may pull gated groups early
                    pop_filler(uncap=(qc == NQC - 1))
                    if m < 3:
                        # async XBAR DMA transposes the [128 q, 128 c] blocks
                        # into the oT[c, q] layout the projection needs --
                        # zero PE/DVE cost; the latency hides behind the rest
                        # of the chunk's attention
                        for sb in range(4):
                            nc.sync.dma_start_transpose(
                                out=oT[:, m, sb * 128 : (sb + 1) * 128],
                                in_=o_sb[:, sb, :],
                            )
                    else:
                        # last pair: its oT gates the next chunk's deferred
                        # output projections, so take the low-latency PE path
                        oT_ps = oT_ps_pool.tile([128, 4, 256], BF16, tag="oTps")
                        for sb in range(4):
                            nc.tensor.matmul(
                                oT_ps[:, sb, 0:128],
                                lhsT=o_sb[:, sb, :],
                                rhs=ident_sb[:],
                                is_transpose=True,
                                start=(sb == 0),
                                stop=(sb == 3),
                                skip_group_check=True,
                            )
                        nc.vector.tensor_copy(
                            out=oT[:, m, :].rearrange("p (s q) -> p s q", s=4),
                            in_=oT_ps[:, :, 0:128],
                        )

            # defer this chunk's output projection into the filler queue
            for tc4 in range(4):
                for ncol in range(2):
                    yq.append(make_y_group(oT, qc, tc4, ncol))
        while yq or gq:
            if yq:
                yq.popleft()()
            else:
                pop_gq()


_MODULE = None


def _get_module():
    global _MODULE
    if _MODULE is None:
        _MODULE = _build_module()
    return _MODULE


def _bf16(a):
    return np.ascontiguousarray(np.asarray(a, dtype=np.float32)).astype(
        ml_dtypes.bfloat16
    )


def _make_in_maps(x, W_qkv, W_proj, b_proj):
    scale = np.float32(1.0 / np.sqrt(HD))
    bias_half = (np.asarray(b_proj, dtype=np.float32) * 0.5).reshape(1, EMB)
    in_maps = []
    for c in range(NCORES):
        b, hg = c // 2, c % 2
        cols = slice(hg * CD, (hg + 1) * CD)
        in_maps.append(
            {
                "xT": _bf16(np.asarray(x[b], dtype=np.float32).T),
                "wq": _bf16(W_qkv[:, 0:EMB][:, cols] * scale),
                "wk": _bf16(W_qkv[:, EMB : 2 * EMB][:, cols]),
                "wv": _bf16(W_qkv[:, 2 * EMB : 3 * EMB][:, cols]),
                "wp": _bf16(W_proj[cols, :]),
                "bias": _bf16(bias_half),
                "ident": np.eye(128, dtype=ml_dtypes.bfloat16),
            }
        )
    return in_maps


def kernel(x, W_qkv, W_proj, b_proj, _trace=False, _trace_kwargs=None):
    x = np.asarray(x, dtype=np.float32)
    W_qkv = np.asarray(W_qkv, dtype=np.float32)
    W_proj = np.asarray(W_proj, dtype=np.float32)
    b_proj = np.asarray(b_proj, dtype=np.float32)

    nc = _get_module()
    in_maps = _make_in_maps(x, W_qkv, W_proj, b_proj)
    res = run_bass_kernel_spmd(
        nc, in_maps, list(range(NCORES)), trace=_trace, **(_trace_kwargs or {})
    )
    out = np.empty((B, S, EMB), dtype=np.float32)
    for b in range(B):
        out[b] = np.asarray(res.results[2 * b]["y"], dtype=np.float32) + np.asarray(
            res.results[2 * b + 1]["y"], dtype=np.float32
        )
    if _trace:
        return out, res
    return out


# revision 45
# speedup vs baseline: 1.0078x; 1.0071x over previous
"""Causal attention block (B=4, S=2048, D=1024, H=16) on 8 Trainium2 NeuronCores.

Sharding: core c = (batch b = c//2, head-group hg = c%2 of 8 heads).
Each core computes QKV projection for its batch restricted to its heads'
columns, causal flash-style attention for its 8 heads, and a partial output
projection (its heads' rows of W_proj). Host sums the two partial outputs
per batch pair and returns the full [4, 2048, 1024] result.

All matmul operands are bf16 (PSUM accumulation stays fp32): on TRN2 the PE
processes one moving row per cycle regardless of dtype, but bf16 halves DMA
bytes, lifts the fp32r moving<256 penalty, and doubles 2-byte DVE ops.

Engine balance: projections (QKV + output) are PE-bound; attention is
Activation-bound (the exp chain). Attention starts as soon as the first
head's q/k columns and the first four key blocks of v exist; every other
projection group lives in an ordered filler queue drained one group per
score-pair iteration (with forced draining to satisfy data dependencies), so
the PE chews projection work whenever the scalar engine is the attention
rate limiter and ideally never idles.

Layout choices (per core):
  - x arrives pre-transposed as xT [1024, 2048] so the embedding dim (the
    matmul contraction dim) is the SBUF partition dim.
  - q, k are produced transposed: qT/kT [512 cols, 2048 tokens] stored as
    [128, 4, 2048] tiles; head h lives in tile chunk h//2, partitions
    (h%2)*64..+64. 1/sqrt(hd) folded into W_q on the host.
  - v is produced in natural [token, col] orientation as [128, 16, 8, 65]
    (key-block, head, 64 v-cols + a ones column for softmax denominators).
  - scores are computed transposed, sT[k, q] = kT_block.T @ qT, into paired
    [128, 2, 512] PSUM tiles so one exp covers two full key blocks (halving
    the activation-engine per-instruction overhead), exp'd with no max
    subtraction (scores are ~N(0,1); fp32 exp cannot overflow), causal
    diagonal masked by a triangular multiply.
  - attention output accumulates in the efficient o[q, d] orientation
    (lhsT = es[k, q-subblock 128], rhs = v[k, 65]): stationary = 128 queries,
    moving = 65, i.e. half the PE rows of the oT[d, q] orientation. The ones
    column yields the denominator as o[:, 64]. PSUM start=True zeroes the
    whole 2KB bank, so only the first write into each bank sets it.
  - normalization is a per-partition tensor_scalar multiply by the
    reciprocal denominator (no partition broadcast needed), packing head
    pairs side by side; a PE transpose of [128, 128] blocks then restores the
    oT[c, q] layout the output projection needs as lhsT.
"""

from collections import deque

import numpy as np
import ml_dtypes

import concourse.bass as bass
import concourse.mybir as mybir
import concourse.tile as tile
from concourse import bacc
from concourse.bass_utils import run_bass_kernel_spmd
from concourse.masks import make_upper_triangular

F32 = mybir.dt.float32
BF16 = mybir.dt.bfloat16
EMB = 1024
HEADS = 16
HD = 64
B = 4
S = 2048
NCORES = 8
HPC = 8           # heads per core
CD = HPC * HD     # 512 cols per core for each of q/k/v
NKB = S // 128    # 16 key blocks
NQC = S // 512    # 4 query chunks

_EXP = mybir.ActivationFunctionType.Exp
_COPY = mybir.ActivationFunctionType.Copy


def _build_module():
    nc = bacc.Bacc("TRN2", target_bir_lowering=False, debug=False)
    xT = nc.declare_dram_parameter("xT", [EMB, S], BF16, isOutput=False)
    wq = nc.declare_dram_parameter("wq", [EMB, CD], BF16, isOutput=False)
    wk = nc.declare_dram_parameter("wk", [EMB, CD], BF16, isOutput=False)
    wv = nc.declare_dram_parameter("wv", [EMB, CD], BF16, isOutput=False)
    wp = nc.declare_dram_parameter("wp", [CD, EMB], BF16, isOutput=False)
    bias = nc.declare_dram_parameter("bias", [1, EMB], BF16, isOutput=False)
    ident = nc.declare_dram_parameter("ident", [128, 128], BF16, isOutput=False)
    y = nc.declare_dram_parameter("y", [S, EMB], BF16, isOutput=True)

    with tile.TileContext(nc) as tc:
        _body(tc, nc, xT, wq, wk, wv, wp, bias, ident, y)
    nc.compile()
    return nc


def _body(tc, nc, xT, wq, wk, wv, wp, bias, ident, y):
    from contextlib import ExitStack

    with ExitStack() as ctx:
        persist = ctx.enter_context(tc.tile_pool(name="persist", bufs=1))
        qt = persist.tile([128, 4, S], BF16, tag="qt")
        kt = persist.tile([128, 4, S], BF16, tag="kt")
        vx = persist.tile([128, NKB, HPC, HD + 1], BF16, tag="vx")
        tri = persist.tile([128, 128], BF16, tag="tri")
        ident_sb = persist.tile([128, 128], BF16, tag="ident")
        wp_sb = persist.tile([128, 4, EMB], BF16, tag="wp")
        bias_sb = persist.tile([128, 1, EMB], BF16, tag="bias")

        # ones column for denominators; causal tri[p, f] = 1.0 iff f >= p
        nc.gpsimd.memset(vx[:, :, :, HD : HD + 1], 1.0)
        make_upper_triangular(nc, tri[:], val=1.0, diag=True)

        xt_pool = ctx.enter_context(tc.tile_pool(name="xt", bufs=2))
        w_pool = ctx.enter_context(tc.tile_pool(name="w", bufs=8))
        wv_pool = ctx.enter_context(tc.tile_pool(name="wvp", bufs=1))
        mm_ps = ctx.enter_context(tc.tile_pool(name="mmps", bufs=2, space="PSUM"))
        s_pool = ctx.enter_context(tc.tile_pool(name="sps", bufs=2, space="PSUM"))
        o_pool = ctx.enter_context(tc.tile_pool(name="ops", bufs=1, space="PSUM"))
        oT_ps_pool = ctx.enter_context(
            tc.tile_pool(name="oTps", bufs=1, space="PSUM")
        )
        e_pool = ctx.enter_context(tc.tile_pool(name="es", bufs=4))
        r_pool = ctx.enter_context(tc.tile_pool(name="recip", bufs=2))
        pair_pool = ctx.enter_context(tc.tile_pool(name="pair", bufs=2))
        oT_pool = ctx.enter_context(tc.tile_pool(name="oT", bufs=2))
        ysb_pool = ctx.enter_context(tc.tile_pool(name="ysb", bufs=4))

        # ---- input loads, spread across the four DMA-issuing engines ----
        # SP: all xT tiles (first-needed first). Act: wq tiles. DVE: wk tiles.
        # Pool: wv, then constants needed only later (ident/wp/bias).
        xt_tiles = {
            0: xt_pool.tile([128, 8, 1024], BF16, tag="xt", name="xt0"),
            1: xt_pool.tile([128, 8, 1024], BF16, tag="xt", name="xt1"),
        }

        def load_xt(half, n2, engine):
            xt_sb = xt_tiles[half]
            for kc in range(8):
                c0 = half * 1024 + n2 * 512
                engine.dma_start(
                    out=xt_sb[:, kc, n2 * 512 : (n2 + 1) * 512],
                    in_=xT[kc * 128 : (kc + 1) * 128, c0 : c0 + 512],
                )

        w_tiles = {}

        def load_w(qk, m, eng, split=False):
            wdram = wq if qk == 0 else wk
            wt = w_pool.tile([128, 8, 128], BF16, tag="w", name=f"w{qk}{m}")
            halves = ((0, 4), (4, 8)) if split else ((0, 8),)
            for c0, c1 in halves:
                eng.dma_start(
                    out=wt[:, c0:c1, :],
                    in_=wdram[
                        c0 * 128 : c1 * 128, m * 128 : (m + 1) * 128
                    ].rearrange("(c p) m -> p c m", p=128),
                )
            w_tiles[(qk, m)] = wt

        # startup: first x quarter split across SP and Act queues, the Act
        # half interleaved with the first head-pairs' projection weights
        xt_sb0 = xt_tiles[0]
        for kc in range(4):
            nc.sync.dma_start(
                out=xt_sb0[:, kc, 0:512], in_=xT[kc * 128 : (kc + 1) * 128, 0:512]
            )
        load_w(0, 0, nc.scalar, split=True)
        for kc in range(4, 6):
            nc.scalar.dma_start(
                out=xt_sb0[:, kc, 0:512], in_=xT[kc * 128 : (kc + 1) * 128, 0:512]
            )
        load_w(1, 0, nc.scalar, split=True)
        for kc in range(6, 8):
            nc.scalar.dma_start(
                out=xt_sb0[:, kc, 0:512], in_=xT[kc * 128 : (kc + 1) * 128, 0:512]
            )
        load_w(0, 1, nc.scalar)
        load_w(1, 1, nc.scalar)
        wv_sb = wv_pool.tile([128, 8, CD], BF16, tag="wv")
        for kc in range(8):
            nc.gpsimd.dma_start(
                out=wv_sb[:, kc, :], in_=wv[kc * 128 : (kc + 1) * 128, :]
            )
        for m in range(2, 4):
            load_w(0, m, nc.scalar)
            load_w(1, m, nc.scalar)
        load_xt(0, 1, nc.gpsimd)
        load_xt(1, 0, nc.sync)
        load_xt(1, 1, nc.sync)
        nc.gpsimd.dma_start(out=ident_sb[:], in_=ident[:])
        nc.gpsimd.dma_start(
            out=wp_sb[:], in_=wp[:].rearrange("(c p) e -> p c e", p=128)
        )
        nc.gpsimd.dma_start(out=bias_sb[:], in_=bias[:].partition_broadcast(128))

        # ---- projection group emitters ----
        def qk_group(half, qk, m, n):
            xt_sb = xt_tiles[half]
            wt = w_tiles[(qk, m)]
            dst = qt if qk == 0 else kt
            ps = mm_ps.tile([128, 512], F32, tag="mmps", name="qkps")
            for kc in range(8):
                nc.tensor.matmul(
                    ps[:],
                    lhsT=(wt[:, kc, :]),
                    rhs=(xt_sb[:, kc, n * 512 : (n + 1) * 512]),
                    start=(kc == 0),
                    stop=(kc == 7),
                )
            col = half * 1024 + n * 512
            nc.vector.tensor_copy(out=dst[:, m, col : col + 512], in_=ps[:])

        def v_group(half, tc8):
            xt_sb = xt_tiles[half]
            tg = half * 8 + tc8
            ps = mm_ps.tile([128, 512], F32, tag="mmps", name="vps")
            for kc in range(8):
                nc.tensor.matmul(
                    ps[:],
                    lhsT=(xt_sb[:, kc, tc8 * 128 : (tc8 + 1) * 128]),
                    rhs=(wv_sb[:, kc, :]),
                    start=(kc == 0),
                    stop=(kc == 7),
                )
            nc.vector.tensor_copy(
                out=vx[:, tg, :, 0:HD],
                in_=ps[:].rearrange("p (h d) -> p h d", h=HPC),
            )

        def make_y_group(oT_prev, qc_prev, tc4, ncol):
            def emit():
                row = qc_prev * 512 + tc4 * 128
                y_ps = mm_ps.tile([128, 512], F32, tag="mmps", name="yps")
                tail = qc_prev == NQC - 1
                for kc in range(4):
                    nc.tensor.matmul(
                        y_ps[:],
                        lhsT=(oT_prev[:, kc, tc4 * 128 : (tc4 + 1) * 128]),
                        rhs=(wp_sb[:, kc, ncol * 512 : (ncol + 1) * 512]),
                        start=(kc == 0),
                        stop=(kc == 3 and not tail),
                        skip_group_check=tail,
                    )
                y_sb = ysb_pool.tile([128, 512], BF16, tag="ysb", name="ysb")
                if qc_prev == NQC - 1:
                    # tail: bias via a 1-partition PE matmul (tri row 0 is
                    # all ones) and copy on the idle Act engine; the DVE
                    # would otherwise serialize the kernel tail
                    nc.tensor.matmul(
                        y_ps[:],
                        lhsT=tri[0:1, :],
                        rhs=bias_sb[0:1, 0, ncol * 512 : (ncol + 1) * 512],
                        start=False,
                        stop=True,
                        skip_group_check=True,
                    )
                    nc.scalar.activation(
                        out=y_sb[:], in_=y_ps[:], func=_COPY
                    )
                else:
                    nc.vector.tensor_add(
                        y_sb[:],
                        y_ps[:],
                        bias_sb[:, 0, ncol * 512 : (ncol + 1) * 512],
                    )
                # last chunk's stores drain at the kernel tail: alternate
                # queues so the final DMAs overlap instead of serializing
                eng = (
                    nc.scalar
                    if qc_prev == NQC - 1 and (2 * tc4 + ncol) % 2
                    else nc.sync
                )
                eng.dma_start(
                    out=y[row : row + 128, ncol * 512 : (ncol + 1) * 512],
                    in_=y_sb[:],
                )

            return emit

        # Ordered projection-group queue: q/k token-slice groups (m-ascending
        # per chunk so heads unblock progressively). gate[(qc, m)] = count
        # that must be emitted before attention chunk qc head-pair m may run.
        # Paced (voluntary) pops are capped below the last chunk's section so
        # that work remains to fill the PE during the Act-bound last chunk.
        # Deferred output projections go to a second queue popped on the
        # pacing slots. v groups are drained at AV-emission granularity.
        G = []
        gate = {}
        for qc in range(NQC):
            half, n = qc // 2, qc % 2
            for m in range(4):
                G.append(("qk", half, 0, m, n))
                G.append(("qk", half, 1, m, n))
                gate[(qc, m)] = len(G)
        gq = deque(G)
        yq = deque()
        drained = [0]
        cur_qc = [0]
        vq = deque((kb // 8, kb % 8) for kb in range(NKB))
        v_drained = [0]

        def pop_gq():
            item = gq.popleft()
            qk_group(item[1], item[2], item[3], item[4])
            drained[0] += 1

        debt = [0.0]  # emitted Act-ns minus emitted PE-ns (cost model est.)

        def pop_filler(reserve=0, uncap=False):
            # per-chunk filler balance: chunk qc's attention consumes the
            # NEXT chunk's q/k groups as filler (the last chunk's are held
            # for its own Act-heavy stretch), and output projections are
            # held until two chunks after they were produced
            qc = cur_qc[0]
            if qc >= 1 and len(yq) > reserve:
                yq.popleft()()
                debt[0] -= 853.0
                return True
            if gq and (uncap or drained[0] < gate[(min(qc + 1, 2), 3)]):
                pop_gq()
                debt[0] -= 1707.0
                return True
            return False

        def pop_while_indebted():
            # keep the PE's emitted work level with the Act engine's: pop
            # filler until the modeled activation debt is covered
            while debt[0] > 0 and pop_filler(reserve=0, uncap=(cur_qc[0] == NQC - 1)):
                pass

        def drain_to(idx):
            while drained[0] < idx:
                pop_gq()

        def drain_v_to(kb_hi):
            while v_drained[0] <= kb_hi and vq:
                half, tc8 = vq.popleft()
                v_group(half, tc8)
                v_drained[0] += 1
                debt[0] -= 1707.0

        # ---------------- attention (Act-bound) + filler drain ----------------
        it = [0]
        for qc in range(NQC):
            cur_qc[0] = qc
            oT = oT_pool.tile([128, 4, 512], BF16, tag="oT")
            kb_max = 4 * qc + 4
            o_sb = None
            for h in range(HPC):
                m, hh = h // 2, h % 2
                drain_to(gate[(qc, m)])
                o_ps = o_pool.tile([128, 4, 128], F32, tag="ops")

                def emit_av(kb, q0, es, jj):
                    sb0 = q0 // 128
                    for sb in range(sb0, 4):
                        nc.tensor.matmul(
                            out=o_ps[:, sb, 0 : HD + 1],
                            lhsT=(
                                es[:, jj, (sb - sb0) * 128 : (sb - sb0 + 1) * 128]
                            ),
                            rhs=(vx[:, kb, h, :]),
                            start=(kb == 0 and sb == 0),
                            stop=(kb == 4 * qc + sb),
                            skip_group_check=True,
                        )

                pending = []
                for pj in range(kb_max // 2):
                    it[0] += 1
                    s_ps = s_pool.tile([128, 2, 512], F32, tag="s")
                    es = e_pool.tile([128, 2, 512], BF16, tag="es")
                    nqs = []
                    for jj in range(2):
                        kb = 2 * pj + jj
                        r = kb * 128 - qc * 512
                        q0 = max(r, 0)
                        nq = 512 - q0
                        nqs.append((kb, q0, nq))
                        nc.tensor.matmul(
                            out=s_ps[:, jj, 0:nq],
                            lhsT=(
                                kt[hh * 64 : hh * 64 + 64, m, kb * 128 : (kb + 1) * 128]
                            ),
                            rhs=(
                                qt[
                                    hh * 64 : hh * 64 + 64,
                                    m,
                                    qc * 512 + q0 : (qc + 1) * 512,
                                ]
                            ),
                            start=True,
                            stop=True,
                        )
                    if nqs[0][2] == 512 and nqs[1][2] == 512:
                        # full pair: one exp over both banks
                        nc.scalar.activation(out=es[:], in_=s_ps[:], func=_EXP)
                        debt[0] += 1024 * 0.833 + 185
                    else:
                        for jj, (kb, q0, nq) in enumerate(nqs):
                            nc.scalar.activation(
                                out=es[:, jj, 0:nq], in_=s_ps[:, jj, 0:nq], func=_EXP
                            )
                            debt[0] += nq * 0.833 + 185
                    debt[0] -= (nqs[0][2] + nqs[1][2]) * 0.4167  # scores
                    for jj, (kb, q0, nq) in enumerate(nqs):
                        if kb * 128 - qc * 512 >= 0:
                            # diagonal block: mask the first 128 query columns
                            nc.vector.tensor_mul(
                                es[:, jj, 0:128], es[:, jj, 0:128], tri[:]
                            )
                    # av matmuls run one pair behind so the PE never waits on
                    # the exp of the pair it just produced
                    if pending:
                        drain_v_to(pending[-1][0])
                    for kb, q0, nq in pending:
                        emit_av(kb, q0, es_prev, kb & 1)
                        debt[0] -= (4 - q0 // 128) * 65 * 0.4167
                    pending, es_prev = nqs, es
                    pop_while_indebted()
                if pending:
                    drain_v_to(pending[-1][0])
                for kb, q0, nq in pending:
                    emit_av(kb, q0, es_prev, kb & 1)
                    debt[0] -= (4 - q0 // 128) * 65 * 0.4167
                pop_while_indebted()
                # normalize: per-partition multiply by 1/denominator
                recip = r_pool.tile([128, 4], F32, tag="recip")
                nc.vector.reciprocal(recip[:], o_ps[:, :, HD])
                if hh == 0:
                    o_sb = pair_pool.tile([128, 4, 128], BF16, tag="pair")
                for sb in range(4):
                    nc.vector.tensor_scalar_mul(
                        o_sb[:, sb, hh * 64 : hh * 64 + 64],
                        o_ps[:, sb, 0:HD],
                        recip[:, sb : sb + 1],
                    )
                if hh == 1:
                    # fill the PE while the normalize chain runs on the DVE;
                    # in the last chunk this may pull gated groups early
                    pop_filler(uncap=(qc == NQC - 1))
                    if m < 3:
                        # async XBAR DMA transposes the [128 q, 128 c] blocks
                        # into the oT[c, q] layout the projection needs --
                        # zero PE/DVE cost; the latency hides behind the rest
                        # of the chunk's attention
                        for sb in range(4):
                            nc.sync.dma_start_transpose(
                                out=oT[:, m, sb * 128 : (sb + 1) * 128],
                                in_=o_sb[:, sb, :],
                            )
                    else:
                        # last pair: its oT gates the next chunk's deferred
                        # output projections, so take the low-latency PE path
                        oT_ps = oT_ps_pool.tile([128, 4, 256], BF16, tag="oTps")
                        for sb in range(4):
                            nc.tensor.matmul(
                                oT_ps[:, sb, 0:128],
                                lhsT=o_sb[:, sb, :],
                                rhs=ident_sb[:],
                                is_transpose=True,
                                start=(sb == 0),
                                stop=(sb == 3),
                                skip_group_check=True,
                            )
                        nc.vector.tensor_copy(
                            out=oT[:, m, :].rearrange("p (s q) -> p s q", s=4),
                            in_=oT_ps[:, :, 0:128],
                        )

            # defer this chunk's output projection into the filler queue
            for tc4 in range(4):
                for ncol in range(2):
                    yq.append(make_y_group(oT, qc, tc4, ncol))
        while yq or gq:
            if yq:
                yq.popleft()()
            else:
                pop_gq()


_MODULE = None


def _get_module():
    global _MODULE
    if _MODULE is None:
        _MODULE = _build_module()
    return _MODULE


def _bf16(a):
    return np.ascontiguousarray(np.asarray(a, dtype=np.float32)).astype(
        ml_dtypes.bfloat16
    )


def _make_in_maps(x, W_qkv, W_proj, b_proj):
    scale = np.float32(1.0 / np.sqrt(HD))
    bias_half = (np.asarray(b_proj, dtype=np.float32) * 0.5).reshape(1, EMB)
    in_maps = []
    for c in range(NCORES):
        b, hg = c // 2, c % 2
        cols = slice(hg * CD, (hg + 1) * CD)
        in_maps.append(
            {
                "xT": _bf16(np.asarray(x[b], dtype=np.float32).T),
                "wq": _bf16(W_qkv[:, 0:EMB][:, cols] * scale),
                "wk": _bf16(W_qkv[:, EMB : 2 * EMB][:, cols]),
                "wv": _bf16(W_qkv[:, 2 * EMB : 3 * EMB][:, cols]),
                "wp": _bf16(W_proj[cols, :]),
                "bias": _bf16(bias_half),
                "ident": np.eye(128, dtype=ml_dtypes.bfloat16),
            }
        )
    return in_maps


def kernel(x, W_qkv, W_proj, b_proj, _trace=False, _trace_kwargs=None):
    x = np.asarray(x, dtype=np.float32)
    W_qkv = np.asarray(W_qkv, dtype=np.float32)
    W_proj = np.asarray(W_proj, dtype=np.float32)
    b_proj = np.asarray(b_proj, dtype=np.float32)

    nc = _get_module()
    in_maps = _make_in_maps(x, W_qkv, W_proj, b_proj)
    res = run_bass_kernel_spmd(
        nc, in_maps, list(range(NCORES)), trace=_trace, **(_trace_kwargs or {})
    )
    out = np.empty((B, S, EMB), dtype=np.float32)
    for b in range(B):
        out[b] = np.asarray(res.results[2 * b]["y"], dtype=np.float32) + np.asarray(
            res.results[2 * b + 1]["y"], dtype=np.float32
        )
    if _trace:
        return out, res
    return out


# revision 46
# speedup vs baseline: 1.0207x; 1.0128x over previous
"""Causal attention block (B=4, S=2048, D=1024, H=16) on 8 Trainium2 NeuronCores.

Sharding: core c = (batch b = c//2, head-group hg = c%2 of 8 heads).
Each core computes QKV projection for its batch restricted to its heads'
columns, causal flash-style attention for its 8 heads, and a partial output
projection (its heads' rows of W_proj). Host sums the two partial outputs
per batch pair and returns the full [4, 2048, 1024] result.

All matmul operands are bf16 (PSUM accumulation stays fp32): on TRN2 the PE
processes one moving row per cycle regardless of dtype, but bf16 halves DMA
bytes, lifts the fp32r moving<256 penalty, and doubles 2-byte DVE ops.

Engine balance: projections (QKV + output) are PE-bound; attention is
Activation-bound (the exp chain). Attention starts as soon as the first
head's q/k columns and the first four key blocks of v exist; every other
projection group lives in an ordered filler queue drained one group per
score-pair iteration (with forced draining to satisfy data dependencies), so
the PE chews projection work whenever the scalar engine is the attention
rate limiter and ideally never idles.

Layout choices (per core):
  - x arrives pre-transposed as xT [1024, 2048] so the embedding dim (the
    matmul contraction dim) is the SBUF partition dim.
  - q, k are produced transposed: qT/kT [512 cols, 2048 tokens] stored as
    [128, 4, 2048] tiles; head h lives in tile chunk h//2, partitions
    (h%2)*64..+64. 1/sqrt(hd) folded into W_q on the host.
  - v is produced in natural [token, col] orientation as [128, 16, 8, 65]
    (key-block, head, 64 v-cols + a ones column for softmax denominators).
  - scores are computed transposed, sT[k, q] = kT_block.T @ qT, into paired
    [128, 2, 512] PSUM tiles so one exp covers two full key blocks (halving
    the activation-engine per-instruction overhead), exp'd with no max
    subtraction (scores are ~N(0,1); fp32 exp cannot overflow), causal
    diagonal masked by a triangular multiply.
  - attention output accumulates in the efficient o[q, d] orientation
    (lhsT = es[k, q-subblock 128], rhs = v[k, 65]): stationary = 128 queries,
    moving = 65, i.e. half the PE rows of the oT[d, q] orientation. The ones
    column yields the denominator as o[:, 64]. PSUM start=True zeroes the
    whole 2KB bank, so only the first write into each bank sets it.
  - normalization is a per-partition tensor_scalar multiply by the
    reciprocal denominator (no partition broadcast needed), packing head
    pairs side by side; a PE transpose of [128, 128] blocks then restores the
    oT[c, q] layout the output projection needs as lhsT.
"""

from collections import deque

import numpy as np
import ml_dtypes

import concourse.bass as bass
import concourse.mybir as mybir
import concourse.tile as tile
from concourse import bacc
from concourse.bass_utils import run_bass_kernel_spmd
from concourse.masks import make_upper_triangular

F32 = mybir.dt.float32
BF16 = mybir.dt.bfloat16
EMB = 1024
HEADS = 16
HD = 64
B = 4
S = 2048
NCORES = 8
HPC = 8           # heads per core
CD = HPC * HD     # 512 cols per core for each of q/k/v
NKB = S // 128    # 16 key blocks
NQC = S // 512    # 4 query chunks

_EXP = mybir.ActivationFunctionType.Exp
_COPY = mybir.ActivationFunctionType.Copy


def _build_module():
    nc = bacc.Bacc("TRN2", target_bir_lowering=False, debug=False)
    xT = nc.declare_dram_parameter("xT", [EMB, S], BF16, isOutput=False)
    wq = nc.declare_dram_parameter("wq", [EMB, CD], BF16, isOutput=False)
    wk = nc.declare_dram_parameter("wk", [EMB, CD], BF16, isOutput=False)
    wv = nc.declare_dram_parameter("wv", [EMB, CD], BF16, isOutput=False)
    wp = nc.declare_dram_parameter("wp", [CD, EMB], BF16, isOutput=False)
    bias = nc.declare_dram_parameter("bias", [1, EMB], BF16, isOutput=False)
    ident = nc.declare_dram_parameter("ident", [128, 128], BF16, isOutput=False)
    y = nc.declare_dram_parameter("y", [S, EMB], BF16, isOutput=True)

    with tile.TileContext(nc) as tc:
        _body(tc, nc, xT, wq, wk, wv, wp, bias, ident, y)
    nc.compile()
    return nc


def _body(tc, nc, xT, wq, wk, wv, wp, bias, ident, y):
    from contextlib import ExitStack

    with ExitStack() as ctx:
        persist = ctx.enter_context(tc.tile_pool(name="persist", bufs=1))
        qt = persist.tile([128, 4, S], BF16, tag="qt")
        kt = persist.tile([128, 4, S], BF16, tag="kt")
        vx = persist.tile([128, NKB, HPC, HD + 1], BF16, tag="vx")
        tri = persist.tile([128, 128], BF16, tag="tri")
        ident_sb = persist.tile([128, 128], BF16, tag="ident")
        wp_sb = persist.tile([128, 4, EMB], BF16, tag="wp")
        bias_sb = persist.tile([128, 1, EMB], BF16, tag="bias")

        # ones column for denominators; causal tri[p, f] = 1.0 iff f >= p
        nc.gpsimd.memset(vx[:, :, :, HD : HD + 1], 1.0)
        make_upper_triangular(nc, tri[:], val=1.0, diag=True)

        xt_pool = ctx.enter_context(tc.tile_pool(name="xt", bufs=2))
        w_pool = ctx.enter_context(tc.tile_pool(name="w", bufs=8))
        wv_pool = ctx.enter_context(tc.tile_pool(name="wvp", bufs=1))
        mm_ps = ctx.enter_context(tc.tile_pool(name="mmps", bufs=2, space="PSUM"))
        s_pool = ctx.enter_context(tc.tile_pool(name="sps", bufs=2, space="PSUM"))
        o_pool = ctx.enter_context(tc.tile_pool(name="ops", bufs=1, space="PSUM"))
        oT_ps_pool = ctx.enter_context(
            tc.tile_pool(name="oTps", bufs=1, space="PSUM")
        )
        e_pool = ctx.enter_context(tc.tile_pool(name="es", bufs=4))
        r_pool = ctx.enter_context(tc.tile_pool(name="recip", bufs=2))
        pair_pool = ctx.enter_context(tc.tile_pool(name="pair", bufs=2))
        oT_pool = ctx.enter_context(tc.tile_pool(name="oT", bufs=2))
        ysb_pool = ctx.enter_context(tc.tile_pool(name="ysb", bufs=4))

        # ---- input loads, spread across the four DMA-issuing engines ----
        # SP: all xT tiles (first-needed first). Act: wq tiles. DVE: wk tiles.
        # Pool: wv, then constants needed only later (ident/wp/bias).
        xt_tiles = {
            0: xt_pool.tile([128, 8, 1024], BF16, tag="xt", name="xt0"),
            1: xt_pool.tile([128, 8, 1024], BF16, tag="xt", name="xt1"),
        }

        def load_xt(half, n2, engine):
            xt_sb = xt_tiles[half]
            for kc in range(8):
                c0 = half * 1024 + n2 * 512
                engine.dma_start(
                    out=xt_sb[:, kc, n2 * 512 : (n2 + 1) * 512],
                    in_=xT[kc * 128 : (kc + 1) * 128, c0 : c0 + 512],
                )

        w_tiles = {}

        def load_w(qk, m, eng, split=False):
            wdram = wq if qk == 0 else wk
            wt = w_pool.tile([128, 8, 128], BF16, tag="w", name=f"w{qk}{m}")
            halves = ((0, 4), (4, 8)) if split else ((0, 8),)
            for c0, c1 in halves:
                eng.dma_start(
                    out=wt[:, c0:c1, :],
                    in_=wdram[
                        c0 * 128 : c1 * 128, m * 128 : (m + 1) * 128
                    ].rearrange("(c p) m -> p c m", p=128),
                )
            w_tiles[(qk, m)] = wt

        # startup: first x quarter split across SP and Act queues, the Act
        # half interleaved with the first head-pairs' projection weights
        xt_sb0 = xt_tiles[0]
        for kc in range(4):
            nc.sync.dma_start(
                out=xt_sb0[:, kc, 0:512], in_=xT[kc * 128 : (kc + 1) * 128, 0:512]
            )
        load_w(0, 0, nc.scalar, split=True)
        for kc in range(4, 6):
            nc.scalar.dma_start(
                out=xt_sb0[:, kc, 0:512], in_=xT[kc * 128 : (kc + 1) * 128, 0:512]
            )
        load_w(1, 0, nc.scalar, split=True)
        for kc in range(6, 8):
            nc.scalar.dma_start(
                out=xt_sb0[:, kc, 0:512], in_=xT[kc * 128 : (kc + 1) * 128, 0:512]
            )
        load_w(0, 1, nc.scalar)
        load_w(1, 1, nc.scalar)
        wv_sb = wv_pool.tile([128, 8, CD], BF16, tag="wv")
        for kc in range(8):
            nc.gpsimd.dma_start(
                out=wv_sb[:, kc, :], in_=wv[kc * 128 : (kc + 1) * 128, :]
            )
        for m in range(2, 4):
            load_w(0, m, nc.scalar)
            load_w(1, m, nc.scalar)
        load_xt(0, 1, nc.gpsimd)
        load_xt(1, 0, nc.sync)
        load_xt(1, 1, nc.sync)
        nc.gpsimd.dma_start(out=ident_sb[:], in_=ident[:])
        nc.gpsimd.dma_start(
            out=wp_sb[:], in_=wp[:].rearrange("(c p) e -> p c e", p=128)
        )
        nc.gpsimd.dma_start(out=bias_sb[:], in_=bias[:].partition_broadcast(128))

        # ---- projection group emitters ----
        def qk_group(half, qk, m, n):
            xt_sb = xt_tiles[half]
            wt = w_tiles[(qk, m)]
            dst = qt if qk == 0 else kt
            ps = mm_ps.tile([128, 512], F32, tag="mmps", name="qkps")
            for kc in range(8):
                nc.tensor.matmul(
                    ps[:],
                    lhsT=(wt[:, kc, :]),
                    rhs=(xt_sb[:, kc, n * 512 : (n + 1) * 512]),
                    start=(kc == 0),
                    stop=(kc == 7),
                )
            col = half * 1024 + n * 512
            nc.vector.tensor_copy(out=dst[:, m, col : col + 512], in_=ps[:])

        def v_group(half, tc8):
            xt_sb = xt_tiles[half]
            tg = half * 8 + tc8
            ps = mm_ps.tile([128, 512], F32, tag="mmps", name="vps")
            for kc in range(8):
                nc.tensor.matmul(
                    ps[:],
                    lhsT=(xt_sb[:, kc, tc8 * 128 : (tc8 + 1) * 128]),
                    rhs=(wv_sb[:, kc, :]),
                    start=(kc == 0),
                    stop=(kc == 7),
                )
            nc.vector.tensor_copy(
                out=vx[:, tg, :, 0:HD],
                in_=ps[:].rearrange("p (h d) -> p h d", h=HPC),
            )

        def make_y_group(oT_prev, qc_prev, tc4, ncol):
            def emit():
                row = qc_prev * 512 + tc4 * 128
                y_ps = mm_ps.tile([128, 512], F32, tag="mmps", name="yps")
                tail = qc_prev == NQC - 1
                for kc in range(4):
                    nc.tensor.matmul(
                        y_ps[:],
                        lhsT=(oT_prev[:, kc, tc4 * 128 : (tc4 + 1) * 128]),
                        rhs=(wp_sb[:, kc, ncol * 512 : (ncol + 1) * 512]),
                        start=(kc == 0),
                        stop=(kc == 3 and not tail),
                        skip_group_check=tail,
                    )
                y_sb = ysb_pool.tile([128, 512], BF16, tag="ysb", name="ysb")
                if qc_prev == NQC - 1:
                    # tail: bias via a 1-partition PE matmul (tri row 0 is
                    # all ones) and copy on the idle Act engine; the DVE
                    # would otherwise serialize the kernel tail
                    nc.tensor.matmul(
                        y_ps[:],
                        lhsT=tri[0:1, :],
                        rhs=bias_sb[0:1, 0, ncol * 512 : (ncol + 1) * 512],
                        start=False,
                        stop=True,
                        skip_group_check=True,
                    )
                    nc.scalar.activation(
                        out=y_sb[:], in_=y_ps[:], func=_COPY
                    )
                else:
                    nc.vector.tensor_add(
                        y_sb[:],
                        y_ps[:],
                        bias_sb[:, 0, ncol * 512 : (ncol + 1) * 512],
                    )
                # last chunk's stores drain at the kernel tail: alternate
                # queues so the final DMAs overlap instead of serializing
                eng = (
                    nc.scalar
                    if qc_prev == NQC - 1 and (2 * tc4 + ncol) % 2
                    else nc.sync
                )
                eng.dma_start(
                    out=y[row : row + 128, ncol * 512 : (ncol + 1) * 512],
                    in_=y_sb[:],
                )

            return emit

        # Ordered projection-group queue: q/k token-slice groups (m-ascending
        # per chunk so heads unblock progressively). gate[(qc, m)] = count
        # that must be emitted before attention chunk qc head-pair m may run.
        # Paced (voluntary) pops are capped below the last chunk's section so
        # that work remains to fill the PE during the Act-bound last chunk.
        # Deferred output projections go to a second queue popped on the
        # pacing slots. v groups are drained at AV-emission granularity.
        G = []
        gate = {}
        for qc in range(NQC):
            half, n = qc // 2, qc % 2
            for m in range(4):
                G.append(("qk", half, 0, m, n))
                G.append(("qk", half, 1, m, n))
                gate[(qc, m)] = len(G)
        gq = deque(G)
        yq = deque()
        drained = [0]
        cur_qc = [0]
        vq = deque((kb // 8, kb % 8) for kb in range(NKB))
        v_drained = [0]

        def pop_gq():
            item = gq.popleft()
            qk_group(item[1], item[2], item[3], item[4])
            drained[0] += 1

        debt = [0.0]  # emitted Act-ns minus emitted PE-ns (cost model est.)

        def pop_filler(reserve=0, uncap=False):
            # per-chunk filler balance: chunk qc's attention consumes the
            # NEXT chunk's q/k groups as filler (the last chunk's are held
            # for its own Act-heavy stretch), and output projections are
            # held until two chunks after they were produced
            qc = cur_qc[0]
            if qc >= 1 and len(yq) > reserve:
                yq.popleft()()
                debt[0] -= 853.0
                return True
            if gq and (uncap or drained[0] < gate[(min(qc + 1, NQC - 1), 3)]):
                pop_gq()
                debt[0] -= 1707.0
                return True
            return False

        def pop_while_indebted():
            # keep the PE's emitted work level with the Act engine's: pop
            # filler until the modeled activation debt is covered
            while debt[0] > 0 and pop_filler(reserve=0, uncap=(cur_qc[0] == NQC - 1)):
                pass

        def drain_to(idx):
            while drained[0] < idx:
                pop_gq()

        def drain_v_to(kb_hi):
            while v_drained[0] <= kb_hi and vq:
                half, tc8 = vq.popleft()
                v_group(half, tc8)
                v_drained[0] += 1
                debt[0] -= 1707.0

        # ---------------- attention (Act-bound) + filler drain ----------------
        it = [0]
        for qc in range(NQC):
            cur_qc[0] = qc
            oT = oT_pool.tile([128, 4, 512], BF16, tag="oT")
            kb_max = 4 * qc + 4
            o_sb = None
            for h in range(HPC):
                m, hh = h // 2, h % 2
                drain_to(gate[(qc, m)])
                o_ps = o_pool.tile([128, 4, 128], F32, tag="ops")

                def emit_av(kb, q0, es, jj):
                    sb0 = q0 // 128
                    for sb in range(sb0, 4):
                        nc.tensor.matmul(
                            out=o_ps[:, sb, 0 : HD + 1],
                            lhsT=(
                                es[:, jj, (sb - sb0) * 128 : (sb - sb0 + 1) * 128]
                            ),
                            rhs=(vx[:, kb, h, :]),
                            start=(kb == 0 and sb == 0),
                            stop=(kb == 4 * qc + sb),
                            skip_group_check=True,
                        )

                pending = []
                for pj in range(kb_max // 2):
                    it[0] += 1
                    s_ps = s_pool.tile([128, 2, 512], F32, tag="s")
                    es = e_pool.tile([128, 2, 512], BF16, tag="es")
                    nqs = []
                    for jj in range(2):
                        kb = 2 * pj + jj
                        r = kb * 128 - qc * 512
                        q0 = max(r, 0)
                        nq = 512 - q0
                        nqs.append((kb, q0, nq))
                        nc.tensor.matmul(
                            out=s_ps[:, jj, 0:nq],
                            lhsT=(
                                kt[hh * 64 : hh * 64 + 64, m, kb * 128 : (kb + 1) * 128]
                            ),
                            rhs=(
                                qt[
                                    hh * 64 : hh * 64 + 64,
                                    m,
                                    qc * 512 + q0 : (qc + 1) * 512,
                                ]
                            ),
                            start=True,
                            stop=True,
                        )
                    if nqs[0][2] == 512 and nqs[1][2] == 512:
                        # full pair: one exp over both banks
                        nc.scalar.activation(out=es[:], in_=s_ps[:], func=_EXP)
                        debt[0] += 1024 * 0.833 + 185
                    else:
                        for jj, (kb, q0, nq) in enumerate(nqs):
                            nc.scalar.activation(
                                out=es[:, jj, 0:nq], in_=s_ps[:, jj, 0:nq], func=_EXP
                            )
                            debt[0] += nq * 0.833 + 185
                    debt[0] -= (nqs[0][2] + nqs[1][2]) * 0.4167  # scores
                    for jj, (kb, q0, nq) in enumerate(nqs):
                        if kb * 128 - qc * 512 >= 0:
                            # diagonal block: mask the first 128 query columns
                            nc.vector.tensor_mul(
                                es[:, jj, 0:128], es[:, jj, 0:128], tri[:]
                            )
                    # av matmuls run one pair behind so the PE never waits on
                    # the exp of the pair it just produced
                    if pending:
                        drain_v_to(pending[-1][0])
                    for kb, q0, nq in pending:
                        emit_av(kb, q0, es_prev, kb & 1)
                        debt[0] -= (4 - q0 // 128) * 65 * 0.4167
                    pending, es_prev = nqs, es
                    pop_while_indebted()
                if pending:
                    drain_v_to(pending[-1][0])
                for kb, q0, nq in pending:
                    emit_av(kb, q0, es_prev, kb & 1)
                    debt[0] -= (4 - q0 // 128) * 65 * 0.4167
                pop_while_indebted()
                # normalize: per-partition multiply by 1/denominator
                recip = r_pool.tile([128, 4], F32, tag="recip")
                nc.vector.reciprocal(recip[:], o_ps[:, :, HD])
                if hh == 0:
                    o_sb = pair_pool.tile([128, 4, 128], BF16, tag="pair")
                for sb in range(4):
                    nc.vector.tensor_scalar_mul(
                        o_sb[:, sb, hh * 64 : hh * 64 + 64],
                        o_ps[:, sb, 0:HD],
                        recip[:, sb : sb + 1],
                    )
                if hh == 1:
                    # fill the PE while the normalize chain runs on the DVE;
                    # in the last chunk this may pull gated groups early
                    pop_filler(uncap=(qc == NQC - 1))
                    if m < 3:
                        # async XBAR DMA transposes the [128 q, 128 c] blocks
                        # into the oT[c, q] layout the projection needs --
                        # zero PE/DVE cost; the latency hides behind the rest
                        # of the chunk's attention
                        for sb in range(4):
                            nc.sync.dma_start_transpose(
                                out=oT[:, m, sb * 128 : (sb + 1) * 128],
                                in_=o_sb[:, sb, :],
                            )
                    else:
                        # last pair: its oT gates the next chunk's deferred
                        # output projections, so take the low-latency PE path
                        oT_ps = oT_ps_pool.tile([128, 4, 256], BF16, tag="oTps")
                        for sb in range(4):
                            nc.tensor.matmul(
                                oT_ps[:, sb, 0:128],
                                lhsT=o_sb[:, sb, :],
                                rhs=ident_sb[:],
                                is_transpose=True,
                                start=(sb == 0),
                                stop=(sb == 3),
                                skip_group_check=True,
                            )
                        nc.vector.tensor_copy(
                            out=oT[:, m, :].rearrange("p (s q) -> p s q", s=4),
                            in_=oT_ps[:, :, 0:128],
                        )

            # defer this chunk's output projection into the filler queue
            for tc4 in range(4):
                for ncol in range(2):
                    yq.append(make_y_group(oT, qc, tc4, ncol))
        while yq or gq:
            if yq:
                yq.popleft()()
            else:
                pop_gq()


_MODULE = None


def _get_module():
    global _MODULE
    if _MODULE is None:
        _MODULE = _build_module()
    return _MODULE


def _bf16(a):
    return np.ascontiguousarray(np.asarray(a, dtype=np.float32)).astype(
        ml_dtypes.bfloat16
    )


def _make_in_maps(x, W_qkv, W_proj, b_proj):
    scale = np.float32(1.0 / np.sqrt(HD))
    bias_half = (np.asarray(b_proj, dtype=np.float32) * 0.5).reshape(1, EMB)
    in_maps = []
    for c in range(NCORES):
        b, hg = c // 2, c % 2
        cols = slice(hg * CD, (hg + 1) * CD)
        in_maps.append(
            {
                "xT": _bf16(np.asarray(x[b], dtype=np.float32).T),
                "wq": _bf16(W_qkv[:, 0:EMB][:, cols] * scale),
                "wk": _bf16(W_qkv[:, EMB : 2 * EMB][:, cols]),
                "wv": _bf16(W_qkv[:, 2 * EMB : 3 * EMB][:, cols]),
                "wp": _bf16(W_proj[cols, :]),
                "bias": _bf16(bias_half),
                "ident": np.eye(128, dtype=ml_dtypes.bfloat16),
            }
        )
    return in_maps


def kernel(x, W_qkv, W_proj, b_proj, _trace=False, _trace_kwargs=None):
    x = np.asarray(x, dtype=np.float32)
    W_qkv = np.asarray(W_qkv, dtype=np.float32)
    W_proj = np.asarray(W_proj, dtype=np.float32)
    b_proj = np.asarray(b_proj, dtype=np.float32)

    nc = _get_module()
    in_maps = _make_in_maps(x, W_qkv, W_proj, b_proj)
    res = run_bass_kernel_spmd(
        nc, in_maps, list(range(NCORES)), trace=_trace, **(_trace_kwargs or {})
    )
    out = np.empty((B, S, EMB), dtype=np.float32)
    for b in range(B):
        out[b] = np.asarray(res.results[2 * b]["y"], dtype=np.float32) + np.asarray(
            res.results[2 * b + 1]["y"], dtype=np.float32
        )
    if _trace:
        return out, res
    return out
